# revision 12
# baseline (speedup 1.0000x reference)
"""HAR 6-channel 2-layer LSTM encoder bank on Trainium2 (Bass/Tile).

Batch-sharded over 8 cores (256 of the 2048 folded sequences each) and
time-chunked within each core: each sequence's 2048 steps are split into
C=8 chunks processed in parallel batch columns, with W=48 warmup steps
(forget-gate decay ~0.5/step makes the carried-in state error < 1e-9).
All 6 LSTM cells (2 layers x 3 encoders, layer 1 lagged one step) are
fused into one [21 x 114] gate matmul per step; i/f/o/g all go through
one sigmoid pass (tanh(g) = 2*sigmoid(2g)-1 recovered on DVE), tanh(c)
is a second scalar-engine pass. Datapath is fp16 (validated 2.0e-3 rel
err vs reference); PSUM/sigmoid internals fp32. Gate groups sit at
partition bases {0,32,64,96}; every 2-tensor DVE op pairs operands at
EQUAL bases (BIR verifier rule), using shifted single-tensor ops to
stage operands where needed.
"""
import sys
import numpy as np

for _p in ('/opt/trn_rl_repo', '/opt/trn_rl_repo/concourse'):
    if _p not in sys.path:
        sys.path.insert(0, _p)

B, T = 1024, 2048
S_ALL = 2 * B            # folded sequences (acc | gyr)
NCORES = 8
S_CORE = S_ALL // NCORES  # 256 sequences per core
C, W = 8, 48              # time chunks per sequence, warmup steps
L = (T - W) // C          # 250; chunk0 covers W+L steps, chunks 1..C-1 cover L
assert W + L + (C - 1) * L == T
STEPS = W + L + 1         # 299 iterations (layer-1 lags one step)
N = C * S_CORE            # 2048 batch columns per core
SLW = min(512, N)         # matmul slice width (PSUM bank = 512 fp32)
NSL = (N + SLW - 1) // SLW

_tG = {'i': 0, 'f': 3, 'g': 6, 'o': 9}
_CB = {'f': 0, 'i': 32, 'o': 64, 'g': 96}  # partition base per gate group


def _pack_weights(inp):
    """Wfull [21, 114] fp16, bias [114,1] fp32.
    rhs rows: 0-8 h layer0 (enc*3+unit), 9-17 h layer1, 18-20 x.
    g-columns x2 (tanh(g) = 2*sigmoid(2g) - 1)."""
    Wf = np.zeros((21, 114), np.float64)
    bias = np.full((114,), -30.0, np.float64)   # junk rows -> sigmoid ~ 0
    for gname, cb in _CB.items():
        toff = _tG[gname]
        gmul = 2.0 if gname == 'g' else 1.0
        for s in range(6):
            layer, e = divmod(s, 3)
            for u in range(3):
                col = cb + s * 3 + u
                if layer == 0:
                    Wf[18:21, col] = gmul * inp['W_ih0'][e, toff + u, :]
                    Wf[e*3:(e+1)*3, col] = gmul * inp['W_hh0'][e, toff + u, :]
                    bias[col] = gmul * (inp['b_ih0'][e, toff + u] + inp['b_hh0'][e, toff + u])
                else:
                    Wf[e*3:(e+1)*3, col] = gmul * inp['W_ih1'][e, toff + u, :]
                    Wf[9+e*3:9+(e+1)*3, col] = gmul * inp['W_hh1'][e, toff + u, :]
                    bias[col] = gmul * (inp['b_ih1'][e, toff + u] + inp['b_hh1'][e, toff + u])
    return Wf.astype(np.float16), bias.astype(np.float32).reshape(114, 1)


def _build_xdev(x):
    """x [B,6,T] fp32 -> x_dev [STEPS, 3, C, S_ALL] fp16 (chunked time map)."""
    xf = np.concatenate([x[:, 0:3, :], x[:, 3:6, :]], axis=0)   # [2B, 3, T]
    xf = np.ascontiguousarray(np.transpose(xf, (1, 2, 0)))      # [3, T, S_ALL]
    xf = np.concatenate([xf, np.zeros((3, 1, S_ALL), xf.dtype)], axis=1)  # pad t=T -> 0
    tidx = np.empty((C, STEPS), np.int64)
    for c in range(C):
        t0 = 0 if c == 0 else (W + L) + (c - 1) * L - W
        tt = t0 + np.arange(STEPS)
        tt[(tt < 0) | (tt >= T)] = T  # zero pad slot
        tidx[c] = tt
    xd = xf[:, tidx, :]                        # [3, C, STEPS, S_ALL]
    xd = np.transpose(xd, (2, 0, 1, 3))        # [STEPS, 3, C, S_ALL]
    return np.ascontiguousarray(xd).astype(np.float16)


def _build_nc():
    import concourse.bacc as bacc
    import concourse.mybir as mybir
    import concourse.tile as tile

    f16 = mybir.dt.float16
    f32 = mybir.dt.float32
    SIG = mybir.ActivationFunctionType.Sigmoid
    TANH = mybir.ActivationFunctionType.Tanh
    MUL = mybir.AluOpType.mult
    SUB = mybir.AluOpType.subtract
    ADD = mybir.AluOpType.add

    nc = bacc.Bacc("TRN2", target_bir_lowering=False, debug=False)
    x_dev = nc.dram_tensor("x_dev", [STEPS, 3, N], f16, kind="ExternalInput").ap()
    wdram = nc.dram_tensor("wt", [21, 114], f16, kind="ExternalInput").ap()
    bdram = nc.dram_tensor("bias", [114, 1], f32, kind="ExternalInput").ap()
    hist = nc.dram_tensor("hist", [STEPS, 9, N], f16, kind="ExternalOutput").ap()

    with tile.TileContext(nc) as tc:
        with (
            tc.tile_pool(name="const", bufs=1) as constp,
            tc.tile_pool(name="state", bufs=3) as sp,
            tc.tile_pool(name="sg", bufs=2) as sgp,
            tc.tile_pool(name="de", bufs=2) as dep,
            tc.tile_pool(name="cv", bufs=2) as cvp,
            tc.tile_pool(name="sc", bufs=2) as scp,
            tc.tile_pool(name="psum", bufs=2, space="PSUM") as pp,
        ):
            wt = constp.tile([21, 114], f16)
            nc.sync.dma_start(wt[:], wdram[:])
            bt = constp.tile([114, 1], f32)
            nc.sync.dma_start(bt[:], bdram[:])
            zt = constp.tile([32, N], f16)
            nc.vector.memset(zt[:], 0.0)

            St = sp.tile([32, N], f16, tag="S")
            nc.vector.memset(St[0:18, :], 0.0)
            nc.sync.dma_start(St[18:21, :], x_dev[0])
            CVprev = cvp.tile([32, N], f16, tag="CV")
            nc.vector.memset(CVprev[0:18, :], 0.0)

            for tau in range(STEPS):
                Snext = sp.tile([32, N], f16, tag="S")
                if tau + 1 < STEPS:
                    nc.sync.dma_start(Snext[18:21, :], x_dev[tau + 1])

                G = pp.tile([128, N], f32, tag="G")
                for s in range(NSL):
                    cs = slice(SLW * s, SLW * (s + 1))
                    nc.tensor.matmul(G[0:114, cs], wt[0:21, :], St[0:21, cs],
                                     start=True, stop=True)
                SG = sgp.tile([128, N], f16, tag="SG")
                nc.scalar.activation(SG[0:114, :], G[0:114, :], SIG, bias=bt[0:114, :])

                U = dep.tile([64, N], f16, tag="U")
                # u = 2*sigma(2g) - 1 = tanh(g); shift out to base 32 to pair with si
                nc.vector.tensor_scalar(U[32:50, :], SG[96:114, :], 2.0, 1.0, MUL, SUB)
                D = dep.tile([32, N], f16, tag="D")
                # d = tanh(g) * si   (equal base 32)
                nc.vector.tensor_tensor(D[0:18, :], U[32:50, :], SG[32:50, :], MUL)
                E = dep.tile([32, N], f16, tag="E")
                # e = c * sf   (equal base 0; gpsimd offload, SBUF-only)
                nc.gpsimd.tensor_tensor(E[0:18, :], CVprev[0:18, :], SG[0:18, :], MUL)
                CV = cvp.tile([32, N], f16, tag="CV")
                # c' = d + e
                nc.vector.tensor_tensor(CV[0:18, :], D[0:18, :], E[0:18, :], ADD)
                SC = scp.tile([96, N], f16, tag="SC")
                # tanh(c), shifted to base 64 to pair with so
                nc.scalar.activation(SC[64:82, :], CV[0:18, :], TANH)
                # h = tanh(c) * so   (equal base 64)
                nc.vector.tensor_tensor(Snext[0:18, :], SC[64:82, :], SG[64:82, :], MUL)
                if tau == 0:
                    # layer-1 state is bias-garbage after the first step: reset
                    nc.sync.dma_start(Snext[9:18, :], zt[0:9, :])
                    nc.sync.dma_start(CV[9:18, :], zt[0:9, :])
                else:
                    nc.sync.dma_start(hist[tau], Snext[9:18, :])
                St, CVprev = Snext, CV
    nc.finalize()
    return nc


_CACHE = {}


def _get_nc():
    if 'nc' not in _CACHE:
        _CACHE['nc'] = _build_nc()
    return _CACHE['nc']


def _run_cores(inp, trace=False):
    from concourse import bass_utils
    nc = _get_nc()
    Wf, bias = _pack_weights(inp)
    xd = _build_xdev(np.asarray(inp['x'], np.float32))  # [STEPS, 3, C, S_ALL]
    in_maps = []
    for k in range(NCORES):
        xk = np.ascontiguousarray(xd[:, :, :, k*S_CORE:(k+1)*S_CORE]).reshape(STEPS, 3, N)
        in_maps.append({'x_dev': xk, 'wt': Wf, 'bias': bias})
    res = bass_utils.run_bass_kernel_spmd(nc, in_maps, core_ids=list(range(NCORES)),
                                          trace=trace)
    return res


def _postprocess(hists, inp):
    """hists: list of 8 arrays [STEPS, 9, N] fp16 (h~1) -> out [B, 6, T] fp32."""
    h1 = np.empty((9, T, S_ALL), np.float32)
    for k in range(NCORES):
        hk = np.asarray(hists[k], np.float32).reshape(STEPS, 9, C, S_CORE)
        cols = slice(k * S_CORE, (k + 1) * S_CORE)
        for c in range(C):
            if c == 0:
                taus = slice(1, W + L + 1); ts = slice(0, W + L)
            else:
                t0 = (W + L) + (c - 1) * L
                taus = slice(W + 1, W + L + 1); ts = slice(t0, t0 + L)
            h1[:, ts, cols] = np.transpose(hk[taus, :, c, :], (1, 0, 2))
    scale = (np.asarray(inp['bn_gamma'], np.float32)
             / np.sqrt(np.asarray(inp['bn_var'], np.float32) + 1e-5))
    mean = np.asarray(inp['bn_mean'], np.float32)
    beta = np.asarray(inp['bn_beta'], np.float32)
    z = (h1.reshape(3, 3, T, S_ALL) - mean[None, None, :, None]) * scale[None, None, :, None] \
        + beta[None, None, :, None]
    re = np.maximum(z, 0.0).mean(axis=0, dtype=np.float32)  # [3, T, S_ALL]
    out = np.empty((B, 6, T), np.float32)
    out[:, 0:3, :] = np.transpose(re[:, :, :B], (2, 0, 1))
    out[:, 3:6, :] = np.transpose(re[:, :, B:], (2, 0, 1))
    return out


def _run(inp, trace=False):
    res = _run_cores(inp, trace=trace)
    hists = [res.results[k]['hist'] for k in range(NCORES)]
    return _postprocess(hists, inp), res


def kernel(**inputs):
    inp = {k: np.asarray(v) for k, v in inputs.items()}
    out, _ = _run(inp)
    return out
